# revision 49
# baseline (speedup 1.0000x reference)
"""Trainium2 Bass kernel for FastWeightMemory (8-core SPMD).

Sharding: chunk-contiguous over the sequence. Core p owns chunks
[8p, 8p+8) (sequence slice [512p, 512p+512) of all 4 batches, 2048
tokens). Per-core phases:

  B: k/v projection per 128-token tile; per-chunk outer products
     accumulate in one persistent PSUM bank (decay scan linearized by
     prescaling v_t with d^-(8p+l+1)/(B*c*|v||k|); the d^-8p factor
     makes the cross-core combine mask-weighted). Outer matmuls lag
     their kv tiles by 2.5 pairs so the norm chain never head-of-line
     blocks the in-order PE queue. Chunk snapshots T_1..T_7 -> bf16;
     T_8 is the collective payload.
  AG: one 8-core AllGather of T'_8 (bf16 128KB), fired immediately at
     end of B.
  C + G_local: q^T projection and r_local = q_l @ T'_l^T overlap the
     collective; r_local evacuates with the d^(8p+l) scale (runtime
     pcf column). Warmup matmuls (zero stationary) then hold the PE
     DVFS clock through the remaining collective wait; PREWARM does
     the same during the initial x-DMA wait.
  post-AG: combine msfc = M0^T + sum_g mask_g * P'_g (vector chain),
     r_corr = q @ msfc (stationary reused, 4 LDWEIGHTS), rt = scale *
     corr + r_local (vector stt / scalar scale-copy + gpsimd add),
     then the out-projection with W_out^T stationary, h-major output
     (host re-transposes), fused per-512-token block with split
     output DMAs so transfers overlap compute.

Hardware facts encoded here: matmul start=True zeroes the ENTIRE
PSUM bank (open each bank once with a zero matmul, accumulate with
start=False); GpSimd cannot touch PSUM and rejects runtime-AP-scalar
ops; matmul duration = out_free * pe_cycle + ~173ns SBUF latency and
LDWEIGHTS pipelines under the previous matmul; the PE clock needs
~3us of continuous work to reach 2.4GHz; only full-8/half/pair
replica groups exist; DMA dependency granularity is whole-tile (size
tiles by consumption order). Inputs are cast to bf16 host-side and
the output is written bf16 (host upcasts); the reference's norm clip
(max_m_norm=10) never activates for this problem's magnitudes, so
the M recurrence is linear.
"""
import sys

for _p in ("/opt/trn_rl_repo", "/root/.axon_site/_ro/trn_rl_repo"):
    if _p not in sys.path:
        sys.path.append(_p)

import numpy as np

import concourse.bass as bass
import concourse.bacc as bacc
import concourse.tile as tile
import concourse.mybir as mybir
from concourse import bass_utils
from concourse.bass_interp import get_hw_module

F32 = mybir.dt.float32
BF16 = mybir.dt.bfloat16
NP_BF16 = mybir.dt.np(BF16)
ALU = mybir.AluOpType
ACT = mybir.ActivationFunctionType

N_CORES = 8
B, S, H, MD = 4, 4096, 1024, 256
CSZ = 64
NCH = S // CSZ          # 64 chunks total
CPC = NCH // N_CORES    # 8 chunks per core
TLOC = CPC * B * CSZ    # 2048 tokens per core
NTT = TLOC // 128       # 16 token tiles
DECAY = 0.99

_BUILT = None
WARMK = 96
PREWARM = 20
DUMMY_AG = False


def _build():
    nc = bacc.Bacc("TRN2", target_bir_lowering=False, debug=False,
                   num_devices=N_CORES)

    # x ts-major: [128 h-low, (ts=16, h_hi=8, c=128)]
    xT = nc.dram_tensor("xT", [128, NTT * 8 * 128], BF16,
                        kind="ExternalInput").ap()
    wkvT = nc.dram_tensor("wkvT", [128, 8 * 2 * MD], BF16,
                          kind="ExternalInput").ap()
    wqT = nc.dram_tensor("wqT", [128, 8 * MD], BF16, kind="ExternalInput").ap()
    woT = nc.dram_tensor("woT", [128, 2 * H], BF16, kind="ExternalInput").ap()
    m0T = nc.dram_tensor("m0T", [128, 2 * MD], F32, kind="ExternalInput").ap()
    pcf = nc.dram_tensor("pcf", [128, 32], F32, kind="ExternalInput").ap()
    # output h-major: [h_tile=8][128 h-low][2048 tok] bf16
    outp = nc.dram_tensor("outp", [8, 128, TLOC], BF16,
                          kind="ExternalOutput").ap()

    with tile.TileContext(nc) as tc, \
         tc.tile_pool(name="persist", bufs=1) as pp:
        x_sb = [pp.tile([128, 2 * 8 * 128], BF16, tag=f"x{i}", name=f"x{i}")
                for i in range(8)]
        wkv_sb = [pp.tile([128, 4 * 2 * MD], BF16, tag=f"wkv{i}",
                          name=f"wkv{i}") for i in range(2)]
        wq_sb = pp.tile([128, 8 * MD], BF16, tag="wq", name="wq")
        wo_sb = pp.tile([128, 2 * H], BF16, tag="wo", name="wo")
        m0_sb = pp.tile([128, 2 * MD], F32, tag="m0", name="m0")
        pcf_sb = pp.tile([128, 32], F32, tag="pcf", name="pcf")
        qT_sb = [pp.tile([128, TLOC], BF16, tag=f"qT{m}", name=f"qT{m}")
                 for m in range(2)]
        # T snapshots: t_sb[j] = S_{j+1}^T (bf16), j=0..6 used by G_local,
        # j=7 is the AG payload
        t_sb = [pp.tile([128, 2 * MD], BF16, tag=f"t{j}", name=f"t{j}")
                for j in range(CPC)]
        ssk = pp.tile([128, NTT], F32, tag="ssk", name="ssk")
        ssv = pp.tile([128, NTT], F32, tag="ssv", name="ssv")
        inv = pp.tile([128, NTT], F32, tag="inv", name="inv")
        pg_sb = [pp.tile([128, 512], BF16, tag=f"pg{i}", name=f"pg{i}")
                 for i in range(9)]
        losum_sb = pp.tile([128, 512], BF16, tag="losum", name="losum")
        accv = pp.tile([128, 2 * MD], F32, tag="accv", name="accv")
        accs = pp.tile([128, 2 * MD], F32, tag="accs", name="accs")
        msfa = pp.tile([128, 2 * MD], BF16, tag="msfa", name="msfa")
        msf = pp.tile([128, 2 * MD], BF16, tag="msf", name="msf")
        msfc = pp.tile([128, 2 * MD], BF16, tag="msfc", name="msfc")
        rt_sb = [pp.tile([128, TLOC], BF16, tag=f"rt{n}", name=f"rt{n}")
                 for n in range(2)]
        rtl_sb = [pp.tile([128, TLOC], BF16, tag=f"rl{n}", name=f"rl{n}")
                  for n in range(2)]
        crt_sb = pp.tile([128, TLOC], BF16, tag="crt", name="crt")
        zt = pp.tile([128, 512], BF16, tag="zt", name="zt")
        outlp_sb = [pp.tile([128, 8 * 512], BF16, tag=f"oT{i}",
                            name=f"oT{i}") for i in range(4)]

        # ---- input DMAs (ts-major x so phase B starts at ~4us) --------
        for qi, qe in enumerate((nc.sync, nc.scalar, nc.gpsimd,
                                 nc.gpsimd)):
            qe.dma_start(wkv_sb[qi // 2][:, (qi % 2) * 1024:
                                         (qi % 2 + 1) * 1024],
                         wkvT[:, qi * 1024:(qi + 1) * 1024])
        qeng = [nc.sync, nc.scalar, nc.gpsimd, nc.sync]
        for i in range(8):
            if i < 2:
                e0, e1 = (nc.sync, nc.scalar) if i == 0 else \
                         (nc.scalar, nc.sync)
                e0.dma_start(x_sb[i][:, 0:1024],
                             xT[:, i * 2048:i * 2048 + 1024])
                e1.dma_start(x_sb[i][:, 1024:2048],
                             xT[:, i * 2048 + 1024:(i + 1) * 2048])
            else:
                eng = [nc.gpsimd, nc.sync, nc.scalar][i % 3]
                eng.dma_start(x_sb[i][:], xT[:, i * 2048:(i + 1) * 2048])
        nc.scalar.dma_start(wq_sb[:], wqT[:])
        nc.scalar.dma_start(wo_sb[:], woT[:])
        nc.gpsimd.dma_start(m0_sb[:], m0T[:])
        nc.gpsimd.dma_start(pcf_sb[:], pcf[:])
        nc.vector.memset(zt[:], 0.0)

        with tc.tile_pool(name="dram", bufs=1, space="DRAM") as dram:
            cin1 = dram.tile([128, 2 * MD], BF16, name="cin1")
            cout1 = dram.tile([N_CORES, 128, 2 * MD], BF16, name="cout1",
                              addr_space="Shared")

            # ---- phase B: kv projection + outer-product scan ---------
            with tc.tile_pool(name="pwork", bufs=6, space="PSUM") as pwork, \
                 tc.tile_pool(name="pT", bufs=2, space="PSUM") as pTp, \
                 tc.tile_pool(name="kvsb", bufs=10) as kvsb, \
                 tc.tile_pool(name="scr", bufs=2) as scr:
                pT_t = [pTp.tile([128, MD], F32, tag="pT",
                                 name=f"pT{mk}") for mk in range(2)]
                # open the T banks once: start=True zeroes the WHOLE
                # bank, so all real outer matmuls use start=False
                for mk in range(2):
                    nc.tensor.matmul(pT_t[mk][:], zt[:, 0:128],
                                     zt[:, 0:MD], start=True, stop=False)
                # pre-B warmup: ramp the PE clock during the x0 DMA
                # wait so phase B starts at 2.4GHz instead of 1.2GHz
                wt0 = pwork.tile([128, 512], F32, tag="pkv", name="warm0")
                for k in range(PREWARM):
                    nc.tensor.matmul(wt0[:], zt[:, 0:128], zt[:],
                                     start=True, stop=True,
                                     skip_group_check=True)
                pend = {}

                def emit_outer(c):
                    kts, vts = pend.pop(c)
                    for tt in range(2):
                        for mk in range(2):
                            nc.tensor.matmul(
                                pT_t[mk][:],
                                kts[tt][:, mk * 128:(mk + 1) * 128],
                                vts[tt][:],
                                start=False,
                                stop=(c == CPC - 1 and tt == 1))
                    # snapshot S_{c+1}^T -> bf16 (both mk banks). The
                    # c==7 snapshot is the collective payload: split it
                    # across vector+scalar so the AG trigger isn't
                    # queued behind scalar's end-of-B norm work.
                    if c == CPC - 1:
                        nc.vector.tensor_copy(t_sb[c][:, 0:MD],
                                              pT_t[0][:])
                        nc.scalar.copy(t_sb[c][:, MD:2 * MD],
                                       pT_t[1][:])
                    else:
                        for mk in range(2):
                            nc.scalar.copy(
                                t_sb[c][:, mk * MD:(mk + 1) * MD],
                                pT_t[mk][:])

                pair_kv = {}
                for tsp in range(NTT // 2):
                    l = tsp
                    xt = x_sb[tsp]
                    pks = [pwork.tile([128, 2 * MD], F32, tag="pkv",
                                      name=f"pkv{j}") for j in range(2)]
                    for hh in range(8):
                        for j in range(2):
                            xoff = j * 1024
                            nc.tensor.matmul(
                                pks[j][:],
                                xt[:, xoff + hh * 128:
                                   xoff + (hh + 1) * 128],
                                wkv_sb[hh // 4][:, (hh % 4) * 2 * MD:
                                                (hh % 4 + 1) * 2 * MD],
                                start=(hh == 0), stop=(hh == 7))
                    kts = []
                    for j in range(2):
                        ts = 2 * tsp + j
                        kt = kvsb.tile([128, MD], BF16, tag="kt",
                                       name="kt")
                        nc.vector.tensor_copy(kt[:], pks[j][:, :MD])
                        sq = scr.tile([128, MD], BF16, tag="sq",
                                      name="sq")
                        nc.vector.scalar_tensor_tensor(
                            sq[:], kt[:], 1.0, kt[:], op0=ALU.mult,
                            op1=ALU.mult, accum_out=ssk[:, ts:ts + 1])
                        sqv = scr.tile([128, MD], BF16, tag="sqv",
                                       name="sqv")
                        nc.scalar.activation(sqv[:], pks[j][:, MD:],
                                             ACT.Square,
                                             accum_out=ssv[:, ts:ts + 1])
                        kts.append(kt)
                    c0 = 2 * tsp
                    nc.vector.tensor_mul(ssk[:, c0:c0 + 2],
                                         ssk[:, c0:c0 + 2],
                                         ssv[:, c0:c0 + 2])
                    nc.scalar.sqrt(ssk[:, c0:c0 + 2], ssk[:, c0:c0 + 2])
                    nc.vector.reciprocal(inv[:, c0:c0 + 2],
                                         ssk[:, c0:c0 + 2])
                    vts = []
                    for j in range(2):
                        t2 = c0 + j
                        vt = kvsb.tile([128, MD], BF16, tag="vt",
                                       name="vt")
                        nc.vector.tensor_scalar(
                            vt[:], pks[j][:, MD:], inv[:, t2:t2 + 1],
                            pcf_sb[:, 10 + l:11 + l],
                            op0=ALU.mult, op1=ALU.mult)
                        vts.append(vt)
                    pend[l] = (kts, vts)
                    if tsp >= 2:
                        emit_outer(tsp - 2)
                for c in range(CPC - 2, CPC):
                    emit_outer(c)

                # ---- fire the AllGather (payload = globally
                # prescaled T'_8 = d^-8p * T_8)
                nc.gpsimd.dma_start(cin1[:], t_sb[CPC - 1][:])
                nc.gpsimd.collective_compute(
                    "AllGather", ALU.bypass,
                    replica_groups=[list(range(N_CORES))],
                    ins=[cin1[:]], outs=[cout1[:]],
                )

                # ---- phase C: q^T projection (overlaps AG) -----------
                for mt in range(2):
                    pq_t = [pwork.tile([128, 512], F32, tag="pkv",
                                       name=f"pq{tb}") for tb in range(4)]
                    for hh in range(8):
                        for sub in range(2):
                            for tb in range(4):
                                xr = x_sb[tb * 2 + sub][:].rearrange(
                                    "p (t h c) -> p t h c", t=2, h=8, c=128)
                                nc.tensor.matmul(
                                    pq_t[tb][:, sub * 256:(sub + 1) * 256],
                                    wq_sb[:, hh * MD + mt * 128:
                                          hh * MD + (mt + 1) * 128],
                                    xr[:, :, hh, :],
                                    start=(hh == 0 and sub == 0),
                                    stop=(hh == 7 and sub == 1))
                    for tb in range(4):
                        if tb % 2 == 0:
                            nc.vector.tensor_copy(
                                qT_sb[mt][:, tb * 512:(tb + 1) * 512],
                                pq_t[tb][:])
                        else:
                            nc.scalar.copy(
                                qT_sb[mt][:, tb * 512:(tb + 1) * 512],
                                pq_t[tb][:])

            # ---- G_local: r_local = q_l @ T_l^T into psum, then evac
            # to rtl with the d^l scale folded (chunk 0 is zero via the
            # bank-opening zero matmul). Banks free before the AG wait
            # so warmup matmuls can hold the PE clock at 2.4GHz.
            with tc.tile_pool(name="pgc", bufs=8, space="PSUM") as pgc:
                ct = [[pgc.tile([128, 512], F32, tag="gc",
                                name=f"gc{nt}_{tb}") for tb in range(4)]
                      for nt in range(2)]
                for nt in range(2):
                    for tb in range(4):
                        nc.tensor.matmul(ct[nt][tb][:], zt[:, 0:128],
                                         zt[:], start=True, stop=False)
                for l in range(1, CPC):
                    for nt in range(2):
                        dst = ct[nt][l // 2][:, (l % 2) * MD:
                                             (l % 2 + 1) * MD]
                        for mk in range(2):
                            nc.tensor.matmul(
                                dst,
                                t_sb[l - 1][:, mk * MD + nt * 128:
                                            mk * MD + (nt + 1) * 128],
                                qT_sb[mk][:, l * MD:(l + 1) * MD],
                                start=False,
                                stop=(l % 2 == 1 and mk == 1))
                for l in range(CPC):
                    for nt in range(2):
                        src_ = ct[nt][l // 2][:, (l % 2) * MD:
                                              (l % 2 + 1) * MD]
                        dst = rtl_sb[nt][:, l * MD:(l + 1) * MD]
                        if nt == 0:
                            nc.vector.tensor_scalar(
                                dst, src_, pcf_sb[:, 18 + l:19 + l],
                                None, op0=ALU.mult)
                        else:
                            nc.scalar.activation(
                                dst, src_, ACT.Copy,
                                scale=pcf_sb[:, 18 + l:19 + l])

                # ---- PE warmup during the AG wait (keeps DVFS high) --
                wt = [pgc.tile([128, 512], F32, tag="gc",
                               name=f"wt{i}") for i in range(2)]
                for k in range(WARMK):
                    nc.tensor.matmul(wt[k % 2][:], zt[:, 0:128], zt[:],
                                     start=True, stop=True,
                                     skip_group_check=True)

                # ---- post-AG: fetch P'_g, mask-weighted combine ------
                dma_engs = [nc.sync, nc.scalar, nc.gpsimd]
                for g in range(N_CORES):
                    dma_engs[g % 3].dma_start(pg_sb[g][:], cout1[g])
                # msfa = M0 + masked g0..3; msf = masked g4..7 (the
                # disjoint halves both feed the corr accumulation; the
                # per-core d^(8p+l) scale is applied at the rt evacs)
                nc.vector.scalar_tensor_tensor(
                    accv[:], pg_sb[0][:], pcf_sb[:, 0:1], m0_sb[:],
                    op0=ALU.mult, op1=ALU.add)
                for g in range(1, 4):
                    dst = accv[:] if g < 3 else msfa[:]
                    nc.vector.scalar_tensor_tensor(
                        dst, pg_sb[g][:], pcf_sb[:, g:g + 1], accv[:],
                        op0=ALU.mult, op1=ALU.add)
                nc.vector.scalar_tensor_tensor(
                    accv[:], pg_sb[4][:], pcf_sb[:, 4:5], zt[:],
                    op0=ALU.mult, op1=ALU.add)
                for g in range(5, N_CORES):
                    dst = accv[:] if g < 7 else msf[:]
                    nc.vector.scalar_tensor_tensor(
                        dst, pg_sb[g][:], pcf_sb[:, g:g + 1], accv[:],
                        op0=ALU.mult, op1=ALU.add)
                nc.vector.tensor_add(msfc[:], msfa[:], msf[:])

                # ---- fused per-token-block pipeline: r_corr (continue
                # + stop the open psum groups) -> rt evac -> out-proj H
                # -> per-block output DMA. H psum tiles recycle the ct
                # pool slots as they drain (1-block lag hides evac
                # latency in the PE queue).
                def corr_block(tb):
                    cc = [pgc.tile([128, 512], F32, tag="gc",
                                   name=f"cc{nt}_{tb}") for nt in range(2)]
                    for mk in range(2):
                        for nt in range(2):
                            nc.tensor.matmul(
                                cc[nt][:],
                                msfc[:, mk * MD + nt * 128:
                                     mk * MD + (nt + 1) * 128],
                                qT_sb[mk][:, tb * 512:(tb + 1) * 512],
                                start=(mk == 0), stop=(mk == 1))
                    # rt = d^l*corr + rtl: nt0 direct on vector; nt1 via
                    # scalar scale-copy + gpsimd add (SBUF-only ops)
                    for l in (2 * tb, 2 * tb + 1):
                        src0 = cc[0][:, (l % 2) * MD:(l % 2 + 1) * MD]
                        nc.vector.scalar_tensor_tensor(
                            rt_sb[0][:, l * MD:(l + 1) * MD],
                            src0, pcf_sb[:, 18 + l:19 + l],
                            rtl_sb[0][:, l * MD:(l + 1) * MD],
                            op0=ALU.mult, op1=ALU.add)
                        src1 = cc[1][:, (l % 2) * MD:(l % 2 + 1) * MD]
                        nc.scalar.activation(
                            crt_sb[:, l * MD:(l + 1) * MD], src1,
                            ACT.Copy, scale=pcf_sb[:, 18 + l:19 + l])
                        nc.gpsimd.tensor_add(
                            rt_sb[1][:, l * MD:(l + 1) * MD],
                            crt_sb[:, l * MD:(l + 1) * MD],
                            rtl_sb[1][:, l * MD:(l + 1) * MD])

                def h_block(lp):
                    ot = outlp_sb[lp]
                    for htp in range(4):
                        hps = [pgc.tile([128, 512], F32, tag="gc",
                                        name=f"hp{lp}_{htp}_{j}")
                               for j in range(2)]
                        for nt in range(2):
                            for j in range(2):
                                ht = 2 * htp + j
                                nc.tensor.matmul(
                                    hps[j][:],
                                    wo_sb[:, nt * H + ht * 128:
                                          nt * H + (ht + 1) * 128],
                                    rt_sb[nt][:, lp * 512:(lp + 1) * 512],
                                    start=(nt == 0), stop=(nt == 1))
                        for j in range(2):
                            ht = 2 * htp + j
                            dst = ot[:, ht * 512:(ht + 1) * 512]
                            if j == 0:
                                nc.vector.tensor_copy(dst, hps[j][:])
                            else:
                                nc.scalar.copy(dst, hps[j][:])
                        ht = 2 * htp + 1
                        if ht == 3:
                            nc.sync.dma_start(
                                outp[0:4, :, lp * 512:(lp + 1) * 512]
                                .rearrange("h p t -> p h t"),
                                ot[:, 0:2048].rearrange(
                                    "p (h t) -> p h t", h=4))
                        elif ht == 7:
                            nc.gpsimd.dma_start(
                                outp[4:8, :, lp * 512:(lp + 1) * 512]
                                .rearrange("h p t -> p h t"),
                                ot[:, 2048:4096].rearrange(
                                    "p (h t) -> p h t", h=4))

                corr_block(0)
                corr_block(1)
                h_block(0)
                corr_block(2)
                h_block(1)
                corr_block(3)
                h_block(2)
                h_block(3)

    nc.compile()
    nc.m = get_hw_module(nc.m)
    return nc


def _get_built():
    global _BUILT
    if _BUILT is None:
        _BUILT = _build()
    return _BUILT


def _to_pm(a, dtype):
    """(n_tiles, 128, F) -> partition-major (128, n_tiles*F)."""
    n, p, f = a.shape
    return np.ascontiguousarray(
        a.transpose(1, 0, 2).reshape(p, n * f)).astype(dtype)


def kernel(x, W_query, W_key, W_value, W_out, M0, chunk_size, **run_kwargs):
    x = np.asarray(x, dtype=np.float32)
    W_query = np.asarray(W_query, dtype=np.float32)
    W_key = np.asarray(W_key, dtype=np.float32)
    W_value = np.asarray(W_value, dtype=np.float32)
    W_out = np.asarray(W_out, dtype=np.float32)
    M0 = np.asarray(M0, dtype=np.float32)
    assert int(chunk_size) == CSZ, f"expected chunk_size {CSZ}"
    assert x.shape == (B, S, H)

    nc = _get_built()

    wq = _to_pm(W_query.T.reshape(8, 128, MD), NP_BF16)
    wkv = _to_pm(np.concatenate(
        [W_key.T.reshape(8, 128, MD), W_value.T.reshape(8, 128, MD)],
        axis=2), NP_BF16)
    wo = _to_pm(W_out.T.reshape(2, 128, H), NP_BF16)
    m0t = _to_pm(M0.T.reshape(2, 128, MD), np.float32)

    in_maps = []
    for p in range(N_CORES):
        xs = x[:, p * 512:(p + 1) * 512, :]
        # token order (chunk, batch, intra-chunk) = (CPC, B, CSZ)
        xs = xs.reshape(B, CPC, CSZ, H).transpose(1, 0, 2, 3)
        xs = xs.reshape(TLOC, H)            # [2048 tok, 1024 h]
        # ts-major: [128 h-low, (ts, h_hi, c)]
        xs = xs.reshape(NTT, 128, 8, 128)   # [ts, c, hh, hl]
        xs = np.ascontiguousarray(
            xs.transpose(3, 0, 2, 1).reshape(128, NTT * 8 * 128)
        ).astype(NP_BF16)
        # cols 0-3: stage-1 slot masks (T'_0..T'_3); col 4: stage-2
        # losum mask; cols 5-8: stage-2 T'_4..T'_7 masks; cols 10-17:
        # vt prescale d^-(l+1)/(B*c) * d^-8p; cols 18-25: rt scale
        # d^(l+8p). Payloads are globally prescaled by d^-8p so the
        # combine is mask-weighted and the per-core factor moves to
        # the rt evacuation scale.
        pc = np.zeros(32, np.float32)
        for g in range(p):
            pc[g] = 1.0
        for l in range(CPC):
            pc[10 + l] = DECAY ** (-(l + 1) - 8 * p) / (B * CSZ)
            pc[18 + l] = DECAY ** (l + 8 * p)
        pcb = np.ascontiguousarray(
            np.broadcast_to(pc, (128, 32)), dtype=np.float32)
        in_maps.append({
            "xT": xs, "wkvT": wkv, "wqT": wq, "woT": wo,
            "m0T": m0t, "pcf": pcb,
        })

    res = bass_utils.run_bass_kernel_spmd(
        nc, in_maps, core_ids=list(range(N_CORES)), **run_kwargs)

    out = np.empty((B, S, H), np.float32)
    for p in range(N_CORES):
        o = res.results[p]["outp"]            # [8, 128, 2048] bf16
        o = o.reshape(H, TLOC).astype(np.float32).T   # [2048 tok, 1024]
        o = o.reshape(CPC, B, CSZ, H).transpose(1, 0, 2, 3)
        out[:, p * 512:(p + 1) * 512, :] = o.reshape(B, 512, H)
    kernel.last_results = res
    return out
